# revision 19
# baseline (speedup 1.0000x reference)
"""Distributed ALiBi causal attention for 8 TRN2 NeuronCores.

Sharding: core c = (b, hg) with b = c // 4 (batch), hg = c % 4 (group of 4
heads = 256 of the 1024 model dims).  Each core:
  1. computes q/k/v projections for its 256-dim column slice of Wq/Wk/Wv
     (transposed layout: qT/kT = W.T @ x.T so the head dim lands on
     partitions),
  2. runs causal attention for its 4 heads in "scores transposed" form
     (scoresT[j, i] = k_j . q_i): the ALiBi bias reduces to a per-key
     -slope*j term (the +slope*i part is constant per softmax row and
     drops out), applied as the per-partition bias of the exp activation
     (shifted by -ln 64 so sums stay in fp16 range; the shift cancels in
     the softmax ratio); softmax denominators come free from a
     ones-column appended to V,
  3. AllGathers the per-head attention outputs across its 4-core batch
     group (chunked by q-range, fp16, overlapped with the next chunk's
     attention), and computes a disjoint 256-column slice of the final
     out projection (contraction over all 16 heads).
The host only slices inputs and concatenates the 8 output slices.

Matmuls run in fp16 (10-bit mantissa = TF32-level accuracy at full PE
rate); accumulation is always fp32 in PSUM.
"""

import math
import os

import numpy as np

B = 2
T = 2048
C = 1024
H = 16
D = 64
N_CORES = 8
HG = 4          # head groups (cores per batch)
HL = 4          # heads per core
DG = HL * D     # 256 d-dims per core
CI = C // 128   # 8 contraction chunks of 128
TB = T // 128   # 16 row blocks of 128
QC = T // 512   # 4 q chunks of 512
KB = T // 128   # 16 key blocks of 128
VROW = D + 1    # v columns per head incl. ones column
LN_SHIFT = float(np.log(64.0))

REPLICA_GROUPS = [[0, 1, 2, 3], [4, 5, 6, 7]]

_COMPILED = None
last_exec_time_ns = None
last_trace_path = None


def _alibi_slopes(n_heads: int) -> np.ndarray:
    def pow2_slopes(n):
        start = 2 ** (-(2 ** (-(math.log2(n) - 3))))
        return [start * start**i for i in range(n)]

    if math.log2(n_heads).is_integer():
        s = pow2_slopes(n_heads)
    else:
        c = 2 ** math.floor(math.log2(n_heads))
        s = pow2_slopes(c)
        s.extend(pow2_slopes(2 * c)[0::2][: n_heads - c])
    return np.array(s, dtype=np.float32)


def _build():
    import concourse.mybir as mybir
    import concourse.tile as tile
    from concourse.alu_op_type import AluOpType
    from concourse.bacc import Bacc
    from contextlib import ExitStack

    F32 = mybir.dt.float32
    F16 = mybir.dt.float16
    ACT = mybir.ActivationFunctionType

    nc = Bacc(None, target_bir_lowering=False, num_devices=N_CORES)

    xT_ext = nc.declare_dram_parameter("xT", [C, T], F16, isOutput=False)
    wq_ext = nc.declare_dram_parameter("wq", [C, DG], F16, isOutput=False)
    wk_ext = nc.declare_dram_parameter("wk", [C, DG], F16, isOutput=False)
    wv_ext = nc.declare_dram_parameter("wv", [C, HL * VROW], F16, isOutput=False)
    wo_ext = nc.declare_dram_parameter("wo", [C, DG], F16, isOutput=False)
    bq_ext = nc.declare_dram_parameter("bq2", [2, 128], F32, isOutput=False)
    bk_ext = nc.declare_dram_parameter("bk2", [2, 128], F32, isOutput=False)
    bfin_ext = nc.declare_dram_parameter("bfin", [1, DG], F16, isOutput=False)
    alibi_ext = nc.declare_dram_parameter("alibi", [128, HL * KB], F32, isOutput=False)
    ones_ext = nc.declare_dram_parameter("ones_", [128, 128], F16, isOutput=False)
    tri_ext = nc.declare_dram_parameter("tri", [128, 128], F16, isOutput=False)
    vb_ext = nc.declare_dram_parameter("vb", [1, HL * VROW], F16, isOutput=False)
    sel_ext = nc.declare_dram_parameter("sel", [HL, 2 * 128], F16, isOutput=False)
    out_ext = nc.declare_dram_parameter("out", [T, DG], F32, isOutput=True)

    with tile.TileContext(nc) as tc, ExitStack() as ctx:
        persist = ctx.enter_context(tc.tile_pool(name="persist", bufs=1))
        wo_sb = persist.tile([128, CI * DG], F16)
        alibi_sb = persist.tile([128, HL * KB], F32)
        bq_sb = persist.tile([128, 2], F32)
        bk_sb = persist.tile([128, 2], F32)
        bfin_sb = persist.tile([1, DG], F16)
        ones_sb = persist.tile([1, 128], F16)
        tri_sb = persist.tile([128, 128], F16)
        vb_sb = persist.tile([1, HL * VROW], F16)
        sel_sb = persist.tile([HL, 2 * 128], F16)

        qkv = ctx.enter_context(tc.tile_pool(name="qkv", bufs=1))
        qT_sb = qkv.tile([128, 2 * T], F16)
        kT_sb = qkv.tile([128, 2 * T], F16)
        v_sb = qkv.tile([128, TB * HL * VROW], F16)
        attn_sb = qkv.tile([128, 2 * T], F16)
        g_sb = qkv.tile([128, CI * T], F16)

        # ---------------- Phase 1: projections -------------------------
        with ExitStack() as p1:
            xw = p1.enter_context(tc.tile_pool(name="xw", bufs=1))
            xT_sb = xw.tile([128, CI * T], F16)
            wq_sb = xw.tile([128, CI * DG], F16)
            wk_sb = xw.tile([128, CI * DG], F16)
            wv_sb = xw.tile([128, CI * HL * VROW], F16)
            for ci in range(CI):
                nc.sync.dma_start(
                    xT_sb[:, ci * T : (ci + 1) * T],
                    xT_ext[ci * 128 : (ci + 1) * 128, :],
                )
                nc.sync.dma_start(
                    wq_sb[:, ci * DG : (ci + 1) * DG],
                    wq_ext[ci * 128 : (ci + 1) * 128, :],
                )
                nc.sync.dma_start(
                    wk_sb[:, ci * DG : (ci + 1) * DG],
                    wk_ext[ci * 128 : (ci + 1) * 128, :],
                )
                nc.sync.dma_start(
                    wv_sb[:, ci * HL * VROW : (ci + 1) * HL * VROW],
                    wv_ext[ci * 128 : (ci + 1) * 128, :],
                )

            nc.sync.dma_start(ones_sb[:], ones_ext[0:1, :])
            nc.sync.dma_start(tri_sb[:], tri_ext[:])
            nc.sync.dma_start(vb_sb[:], vb_ext[:])
            nc.sync.dma_start(sel_sb[:], sel_ext[:])
            nc.sync.dma_start(alibi_sb[:], alibi_ext[:])
            nc.sync.dma_start(bfin_sb[:], bfin_ext[:])
            for db in range(2):
                nc.sync.dma_start(bq_sb[:, db : db + 1], bq_ext[db : db + 1, :])
                nc.sync.dma_start(bk_sb[:, db : db + 1], bk_ext[db : db + 1, :])
            for ci in range(CI):
                nc.sync.dma_start(
                    wo_sb[:, ci * DG : (ci + 1) * DG],
                    wo_ext[ci * 128 : (ci + 1) * 128, :],
                )

            pps = p1.enter_context(tc.tile_pool(name="pps", bufs=3, space="PSUM"))

            # qT / kT: [256 d, 2048 t] as 2 partition blocks of 128
            for w_sb, t_sb, b_sb in ((wq_sb, qT_sb, bq_sb), (wk_sb, kT_sb, bk_sb)):
                for db in range(2):
                    for qc in range(QC):
                        ps = pps.tile([128, 512], F32, tag="proj")
                        for ci in range(CI):
                            nc.tensor.matmul(
                                ps[:],
                                w_sb[:, ci * DG + db * 128 : ci * DG + db * 128 + 128],
                                xT_sb[:, ci * T + qc * 512 : ci * T + qc * 512 + 512],
                                start=(ci == 0),
                                stop=(ci == CI - 1),
                            )
                        nc.scalar.activation(
                            t_sb[:, db * T + qc * 512 : db * T + qc * 512 + 512],
                            ps[:],
                            ACT.Identity,
                            bias=b_sb[:, db : db + 1],
                        )

            # v in augmented layout [t, 4 heads x (64 dims + ones col)]:
            # Wv is host-padded with zero columns at the ones slots and the
            # constant 1.0 comes from a K=1 bias matmul, so every write into
            # v_sb is a contiguous [128, 260] copy.
            for tb in range(TB):
                ps = pps.tile([128, HL * VROW], F32, tag="vproj")
                for ci in range(CI):
                    nc.tensor.matmul(
                        ps[:],
                        xT_sb[:, ci * T + tb * 128 : ci * T + tb * 128 + 128],
                        wv_sb[:, ci * HL * VROW : (ci + 1) * HL * VROW],
                        start=(ci == 0),
                        stop=False,
                    )
                nc.tensor.matmul(
                    ps[:], ones_sb[0:1, :], vb_sb[:], start=False, stop=True
                )
                nc.vector.tensor_copy(
                    v_sb[:, tb * HL * VROW : (tb + 1) * HL * VROW], ps[:]
                )

        # -------- Phase 2+3+4: attention / AllGather / out-proj --------
        # interleaved per q-chunk so the collective overlaps compute
        dram = ctx.enter_context(tc.tile_pool(name="dram", bufs=1, space="DRAM"))
        with ExitStack() as p2:
            qk_ps = p2.enter_context(tc.tile_pool(name="qk_ps", bufs=2, space="PSUM"))
            av_ps = p2.enter_context(tc.tile_pool(name="av_ps", bufs=1, space="PSUM"))
            bc_ps = p2.enter_context(tc.tile_pool(name="bc_ps", bufs=1, space="PSUM"))
            wo_ps = p2.enter_context(tc.tile_pool(name="wo_ps", bufs=1, space="PSUM"))
            expp = p2.enter_context(tc.tile_pool(name="expp", bufs=6))
            nrm = p2.enter_context(tc.tile_pool(name="nrm", bufs=2))
            outp = p2.enter_context(tc.tile_pool(name="outp", bufs=3))

            def emit_wo_tb(tb):
                wp = wo_ps.tile([128, DG], F32, tag="wo", name=f"wp{tb}")
                for ci in range(CI):
                    nc.tensor.matmul(
                        wp[:],
                        g_sb[:, ci * T + tb * 128 : ci * T + tb * 128 + 128],
                        wo_sb[:, ci * DG : (ci + 1) * DG],
                        start=(ci == 0),
                        stop=False,
                    )
                nc.tensor.matmul(
                    wp[:], ones_sb[0:1, :], bfin_sb[:], start=False, stop=True
                )
                ot = outp.tile([128, DG], F32, tag="out", name=f"ot{tb}")
                nc.vector.tensor_copy(ot[:], wp[:])
                nc.sync.dma_start(out_ext[tb * 128 : (tb + 1) * 128, :], ot[:])

            qc_order = [QC - 1 - i for i in range(QC)]
            prev_qc = None
            for qc in qc_order:
                dstrip = nrm.tile([65, HL * 512], F32, tag="dstrip")
                nkb = 4 * (qc + 1)
                for hp in range(2):
                    h0, h1 = 2 * hp, 2 * hp + 1
                    q0 = qT_sb[0:64, hp * T + qc * 512 : hp * T + qc * 512 + 512]
                    q1 = qT_sb[64:128, hp * T + qc * 512 : hp * T + qc * 512 + 512]
                    av0 = av_ps.tile([VROW, 512], F32, tag="av0", name="av0")
                    av1 = av_ps.tile([VROW, 512], F32, tag="av1", name="av1")
                    LAG = 2
                    ets = {}

                    def emit_av(kb):
                        r = kb - 4 * qc
                        c0 = 128 * r if r > 0 else 0
                        e0, e1 = ets.pop(kb)
                        for h, avp, et in ((h0, av0, e0), (h1, av1, e1)):
                            nc.tensor.matmul(
                                avp[:, c0:512],
                                v_sb[
                                    :,
                                    kb * HL * VROW + h * VROW : kb * HL * VROW
                                    + (h + 1) * VROW,
                                ],
                                et[:, c0:512],
                                start=(kb == 0),
                                stop=(kb == nkb - 1),
                            )

                    for kb in range(nkb + LAG):
                        if kb < nkb:
                            r = kb - 4 * qc
                            c0 = 128 * r if r > 0 else 0
                            # two heads ride different PE row groups: their
                            # matmuls run concurrently and each LDWEIGHTS
                            # overlaps the sibling's matmul
                            qkp0 = qk_ps.tile([128, 512], F32, tag="qk0", name="qkp0")
                            qkp1 = qk_ps.tile([128, 512], F32, tag="qk1", name="qkp1")
                            nc.tensor.matmul(
                                qkp0[:, c0:512],
                                kT_sb[
                                    0:64,
                                    hp * T + kb * 128 : hp * T + kb * 128 + 128,
                                ],
                                q0[:, c0:512],
                                start=True,
                                stop=True,
                            )
                            nc.tensor.matmul(
                                qkp1[:, c0:512],
                                kT_sb[
                                    64:128,
                                    hp * T + kb * 128 : hp * T + kb * 128 + 128,
                                ],
                                q1[:, c0:512],
                                start=True,
                                stop=True,
                            )
                            e0 = expp.tile([128, 512], F16, tag="exp", name="e0")
                            e1 = expp.tile([128, 512], F16, tag="exp", name="e1")
                            ets[kb] = (e0, e1)
                            for h, qkp, et in ((h0, qkp0, e0), (h1, qkp1, e1)):
                                nc.scalar.activation(
                                    et[:, c0:512],
                                    qkp[:, c0:512],
                                    ACT.Exp,
                                    bias=alibi_sb[:, h * KB + kb : h * KB + kb + 1],
                                    scale=float(D) ** -0.5,
                                )
                                if r >= 0:
                                    nc.vector.tensor_tensor(
                                        et[:, c0 : c0 + 128],
                                        et[:, c0 : c0 + 128],
                                        tri_sb[:],
                                        AluOpType.mult,
                                    )
                        if kb >= LAG:
                            emit_av(kb - LAG)

                    # denominators + unnormalized numerators, then one row
                    # block of the previous chunk's out projection per head
                    for h, avp in ((h0, av0), (h1, av1)):
                        base = (h % 2) * 64
                        d_sl = dstrip[64:65, h * 512 : (h + 1) * 512]
                        a_sl = attn_sb[
                            base : base + 64,
                            hp * T + qc * 512 : hp * T + qc * 512 + 512,
                        ]
                        nc.vector.tensor_copy(d_sl, avp[D : D + 1, :])
                        nc.vector.tensor_copy(a_sl, avp[0:D, :])
                        if prev_qc is not None:
                            emit_wo_tb(4 * prev_qc + h)

                # one batched reciprocal per chunk (DVE): starts while the
                # previous chunk's out-projection keeps the PE busy
                rall = nrm.tile([HL, 512], F32, tag="rall")
                nc.sync.dma_start(
                    rall[:],
                    dstrip[64:65, :].rearrange("p (h x) -> p h x", h=HL),
                )
                rrec = nrm.tile([HL, 512], F32, tag="rrec")
                nc.vector.reciprocal(rrec[:], rall[:])
                rr16 = nrm.tile([HL, 512], F16, tag="rr16")
                nc.vector.tensor_copy(rr16[:], rrec[:])

                for hcol in range(2):
                    bcp = bc_ps.tile([128, 512], F32, tag="bc")
                    nc.tensor.matmul(
                        bcp[:],
                        sel_sb[:, hcol * 128 : (hcol + 1) * 128],
                        rr16[:],
                        start=True,
                        stop=True,
                    )
                    bcs = nrm.tile([128, 512], F16, tag="bcs")
                    nc.vector.tensor_copy(bcs[:], bcp[:])
                    a_sl = attn_sb[
                        :, hcol * T + qc * 512 : hcol * T + qc * 512 + 512
                    ]
                    nc.vector.tensor_tensor(a_sl, a_sl, bcs[:], AluOpType.mult)

                # AllGather this q-chunk across the 4-core batch group
                attn_dram = dram.tile([DG, 512], F16, tag=f"ad{qc}", name=f"ad{qc}")
                gathered = dram.tile(
                    [HG * DG, 512], F16, tag=f"gd{qc}", name=f"gd{qc}"
                )
                for db in range(2):
                    nc.sync.dma_start(
                        attn_dram[db * 128 : (db + 1) * 128, :],
                        attn_sb[:, db * T + qc * 512 : db * T + qc * 512 + 512],
                    )
                nc.gpsimd.collective_compute(
                    "AllGather",
                    mybir.AluOpType.bypass,
                    replica_groups=REPLICA_GROUPS,
                    ins=[attn_dram[:].opt()],
                    outs=[gathered[:].opt()],
                )
                for ci in range(CI):
                    nc.sync.dma_start(
                        g_sb[:, ci * T + qc * 512 : ci * T + qc * 512 + 512],
                        gathered[ci * 128 : (ci + 1) * 128, :],
                    )
                prev_qc = qc
            for tb in range(4 * prev_qc, 4 * prev_qc + 4):
                emit_wo_tb(tb)

    nc.compile()
    return nc


def _get_compiled():
    global _COMPILED
    if _COMPILED is None:
        _COMPILED = _build()
    return _COMPILED


def _make_in_maps(x, Wq, bq, Wk, bk, Wv, bv, Wo, bo):
    slopes = _alibi_slopes(H)
    ones128 = np.ones((128, 128), dtype=np.float16)
    # tri[p, f] = 1 where key-offset p <= q-offset f (causal keep region)
    tri = np.triu(np.ones((128, 128), dtype=np.float16))
    sel = np.zeros((HL, 2 * 128), dtype=np.float16)
    for c_ in range(2):
        sel[2 * c_, c_ * 128 : c_ * 128 + 64] = 1.0
        sel[2 * c_ + 1, c_ * 128 + 64 : c_ * 128 + 128] = 1.0
    vbias = np.zeros((1, HL * VROW), dtype=np.float16)
    for h in range(HL):
        vbias[0, h * VROW + D] = 1.0
    in_maps = []
    for c in range(N_CORES):
        b, hg = divmod(c, HG)
        sl = slice(hg * DG, (hg + 1) * DG)
        alibi = np.empty((128, HL * KB), dtype=np.float32)
        p = np.arange(128, dtype=np.float32)[:, None]
        for h in range(HL):
            s = slopes[hg * HL + h]
            kbs = np.arange(KB, dtype=np.float32)[None, :]
            alibi[:, h * KB : (h + 1) * KB] = -s * (128.0 * kbs + p) - LN_SHIFT
        bfin = (bv @ Wo[:, sl] + bo[sl]).astype(np.float16).reshape(1, DG)
        wv_pad = np.zeros((C, HL * VROW), dtype=np.float16)
        for h in range(HL):
            wv_pad[:, h * VROW : h * VROW + D] = Wv[:, hg * DG + h * D : hg * DG + (h + 1) * D].astype(np.float16)
        in_maps.append(
            {
                "xT": np.ascontiguousarray(x[b].T).astype(np.float16),
                "wq": np.ascontiguousarray(Wq[:, sl]).astype(np.float16),
                "wk": np.ascontiguousarray(Wk[:, sl]).astype(np.float16),
                "wv": wv_pad,
                "wo": np.ascontiguousarray(Wo[:, sl]).astype(np.float16),
                "bq2": np.ascontiguousarray(bq[sl].reshape(2, 128)).astype(np.float32),
                "bk2": np.ascontiguousarray(bk[sl].reshape(2, 128)).astype(np.float32),
                "bfin": bfin,
                "alibi": alibi,
                "ones_": ones128,
                "tri": tri,
                "vb": vbias,
                "sel": sel,
            }
        )
    return in_maps


def kernel(x, Wq, bq, Wk, bk, Wv, bv, Wo, bo):
    global last_exec_time_ns, last_trace_path
    x = np.asarray(x, dtype=np.float32)
    Wq = np.asarray(Wq, dtype=np.float32)
    bq = np.asarray(bq, dtype=np.float32)
    Wk = np.asarray(Wk, dtype=np.float32)
    bk = np.asarray(bk, dtype=np.float32)
    Wv = np.asarray(Wv, dtype=np.float32)
    bv = np.asarray(bv, dtype=np.float32)
    Wo = np.asarray(Wo, dtype=np.float32)
    bo = np.asarray(bo, dtype=np.float32)

    from concourse import bass_utils

    trace = bool(os.environ.get("BASS_KERNEL_TRACE"))
    kwargs = {}
    if trace:
        try:
            import sys
            import types

            import antenv

            if "antenv.axon_hooks" not in sys.modules:
                hooks = types.ModuleType("antenv.axon_hooks")
                _h = [None]
                hooks.set_axon_ntff_profile_hook = lambda fn: _h.__setitem__(0, fn)
                hooks.get_axon_ntff_profile_hook = lambda: _h[0]
                sys.modules["antenv.axon_hooks"] = hooks
                antenv.axon_hooks = hooks
                from trn_agent_boot.trn_boot import _ntff_profile_via_ctypes

                hooks.set_axon_ntff_profile_hook(
                    _ntff_profile_via_ctypes("/opt/axon/libaxon_pjrt.so")
                )
            bass_utils.upload_artifacts = lambda tmpdir: "local://" + str(tmpdir)
            kwargs = {"trace": True, "tmpdir": os.environ.get("BASS_KERNEL_TRACE_DIR")}
        except Exception as e:  # pragma: no cover
            print(f"trace setup failed ({e}); running untraced")
            trace = False

    nc = _get_compiled()
    in_maps = _make_in_maps(x, Wq, bq, Wk, bk, Wv, bv, Wo, bo)
    res = bass_utils.run_bass_kernel_spmd(
        nc, in_maps, core_ids=list(range(N_CORES)), **kwargs
    )
    if trace:
        last_exec_time_ns = res.exec_time_ns
        if res.instructions_and_trace is not None:
            last_trace_path = res.instructions_and_trace[1]

    out = np.empty((B, T, C), dtype=np.float32)
    for c in range(N_CORES):
        b, hg = divmod(c, HG)
        out[b, :, hg * DG : (hg + 1) * DG] = res.results[c]["out"]
    return out


# revision 20
# speedup vs baseline: 1.1065x; 1.1065x over previous
"""Distributed ALiBi causal attention for 8 TRN2 NeuronCores.

Sharding: core c = (b, hg) with b = c // 4 (batch), hg = c % 4 (group of 4
heads = 256 of the 1024 model dims).  Each core:
  1. computes q/k/v projections for its 256-dim column slice of Wq/Wk/Wv
     (transposed layout: qT/kT = W.T @ x.T so the head dim lands on
     partitions),
  2. runs causal attention for its 4 heads in "scores transposed" form
     (scoresT[j, i] = k_j . q_i): the ALiBi bias reduces to a per-key
     -slope*j term (the +slope*i part is constant per softmax row and
     drops out), applied as the per-partition bias of the exp activation
     (shifted by -ln 64 so sums stay in fp16 range; the shift cancels in
     the softmax ratio); softmax denominators come free from a
     ones-column appended to V,
  3. AllGathers the per-head attention outputs across its 4-core batch
     group (chunked by q-range, fp16, overlapped with the next chunk's
     attention), and computes a disjoint 256-column slice of the final
     out projection (contraction over all 16 heads).
The host only slices inputs and concatenates the 8 output slices.

Matmuls run in fp16 (10-bit mantissa = TF32-level accuracy at full PE
rate); accumulation is always fp32 in PSUM.
"""

import math
import os

import numpy as np

B = 2
T = 2048
C = 1024
H = 16
D = 64
N_CORES = 8
HG = 4          # head groups (cores per batch)
HL = 4          # heads per core
DG = HL * D     # 256 d-dims per core
CI = C // 128   # 8 contraction chunks of 128
TB = T // 128   # 16 row blocks of 128
QC = T // 512   # 4 q chunks of 512
KB = T // 128   # 16 key blocks of 128
VROW = D + 1    # v columns per head incl. ones column
LN_SHIFT = float(np.log(64.0))

REPLICA_GROUPS = [[0, 1, 2, 3], [4, 5, 6, 7]]

_COMPILED = None
last_exec_time_ns = None
last_trace_path = None


def _alibi_slopes(n_heads: int) -> np.ndarray:
    def pow2_slopes(n):
        start = 2 ** (-(2 ** (-(math.log2(n) - 3))))
        return [start * start**i for i in range(n)]

    if math.log2(n_heads).is_integer():
        s = pow2_slopes(n_heads)
    else:
        c = 2 ** math.floor(math.log2(n_heads))
        s = pow2_slopes(c)
        s.extend(pow2_slopes(2 * c)[0::2][: n_heads - c])
    return np.array(s, dtype=np.float32)


def _build():
    import concourse.mybir as mybir
    import concourse.tile as tile
    from concourse.alu_op_type import AluOpType
    from concourse.bacc import Bacc
    from contextlib import ExitStack

    F32 = mybir.dt.float32
    F16 = mybir.dt.float16
    ACT = mybir.ActivationFunctionType

    nc = Bacc(None, target_bir_lowering=False, num_devices=N_CORES)

    xT_ext = nc.declare_dram_parameter("xT", [C, T], F16, isOutput=False)
    wq_ext = nc.declare_dram_parameter("wq", [C, DG], F16, isOutput=False)
    wk_ext = nc.declare_dram_parameter("wk", [C, DG], F16, isOutput=False)
    wv_ext = nc.declare_dram_parameter("wv", [C, HL * VROW], F16, isOutput=False)
    wo_ext = nc.declare_dram_parameter("wo", [C, DG], F16, isOutput=False)
    bq_ext = nc.declare_dram_parameter("bq2", [2, 128], F32, isOutput=False)
    bk_ext = nc.declare_dram_parameter("bk2", [2, 128], F32, isOutput=False)
    bfin_ext = nc.declare_dram_parameter("bfin", [1, DG], F16, isOutput=False)
    alibi_ext = nc.declare_dram_parameter("alibi", [128, HL * KB], F32, isOutput=False)
    ones_ext = nc.declare_dram_parameter("ones_", [128, 128], F16, isOutput=False)
    tri_ext = nc.declare_dram_parameter("tri", [128, 128], F16, isOutput=False)
    vb_ext = nc.declare_dram_parameter("vb", [1, HL * VROW], F16, isOutput=False)
    sel_ext = nc.declare_dram_parameter("sel", [2, 128], F16, isOutput=False)
    out_ext = nc.declare_dram_parameter("out", [T, DG], F32, isOutput=True)

    with tile.TileContext(nc) as tc, ExitStack() as ctx:
        persist = ctx.enter_context(tc.tile_pool(name="persist", bufs=1))
        wo_sb = persist.tile([128, CI * DG], F16)
        alibi_sb = persist.tile([128, HL * KB], F32)
        bq_sb = persist.tile([128, 2], F32)
        bk_sb = persist.tile([128, 2], F32)
        bfin_sb = persist.tile([1, DG], F16)
        ones_sb = persist.tile([1, 128], F16)
        tri_sb = persist.tile([128, 128], F16)
        vb_sb = persist.tile([1, HL * VROW], F16)
        sel_sb = persist.tile([2, 128], F16)

        qkv = ctx.enter_context(tc.tile_pool(name="qkv", bufs=1))
        qT_sb = qkv.tile([128, 2 * T], F16)
        kT_sb = qkv.tile([128, 2 * T], F16)
        v_sb = qkv.tile([128, TB * HL * VROW], F16)
        attn_sb = qkv.tile([128, 2 * T], F16)
        g_sb = qkv.tile([128, CI * T], F16)

        # ---------------- Phase 1: projections -------------------------
        with ExitStack() as p1:
            xw = p1.enter_context(tc.tile_pool(name="xw", bufs=1))
            xT_sb = xw.tile([128, CI * T], F16)
            wq_sb = xw.tile([128, CI * DG], F16)
            wk_sb = xw.tile([128, CI * DG], F16)
            wv_sb = xw.tile([128, CI * HL * VROW], F16)
            for ci in range(CI):
                nc.sync.dma_start(
                    xT_sb[:, ci * T : (ci + 1) * T],
                    xT_ext[ci * 128 : (ci + 1) * 128, :],
                )
                nc.sync.dma_start(
                    wq_sb[:, ci * DG : (ci + 1) * DG],
                    wq_ext[ci * 128 : (ci + 1) * 128, :],
                )
                nc.sync.dma_start(
                    wk_sb[:, ci * DG : (ci + 1) * DG],
                    wk_ext[ci * 128 : (ci + 1) * 128, :],
                )
                nc.sync.dma_start(
                    wv_sb[:, ci * HL * VROW : (ci + 1) * HL * VROW],
                    wv_ext[ci * 128 : (ci + 1) * 128, :],
                )

            nc.sync.dma_start(ones_sb[:], ones_ext[0:1, :])
            nc.sync.dma_start(tri_sb[:], tri_ext[:])
            nc.sync.dma_start(vb_sb[:], vb_ext[:])
            nc.sync.dma_start(sel_sb[:], sel_ext[:])
            nc.sync.dma_start(alibi_sb[:], alibi_ext[:])
            nc.sync.dma_start(bfin_sb[:], bfin_ext[:])
            for db in range(2):
                nc.sync.dma_start(bq_sb[:, db : db + 1], bq_ext[db : db + 1, :])
                nc.sync.dma_start(bk_sb[:, db : db + 1], bk_ext[db : db + 1, :])
            for ci in range(CI):
                nc.sync.dma_start(
                    wo_sb[:, ci * DG : (ci + 1) * DG],
                    wo_ext[ci * 128 : (ci + 1) * 128, :],
                )

            pps = p1.enter_context(tc.tile_pool(name="pps", bufs=3, space="PSUM"))

            # qT / kT: [256 d, 2048 t] as 2 partition blocks of 128.
            # kT first in key order, then qT starting with the last q-chunk:
            # attention processes chunks in reverse order.
            def proj_ranges():
                for qc in range(QC):
                    yield wk_sb, kT_sb, bk_sb, 0, qc
                    yield wk_sb, kT_sb, bk_sb, 1, qc
                for qc in reversed(range(QC)):
                    yield wq_sb, qT_sb, bq_sb, 0, qc
                    yield wq_sb, qT_sb, bq_sb, 1, qc

            for w_sb, t_sb, b_sb, db, qc in proj_ranges():
                if True:
                    if True:
                        ps = pps.tile([128, 512], F32, tag="proj")
                        for ci in range(CI):
                            nc.tensor.matmul(
                                ps[:],
                                w_sb[:, ci * DG + db * 128 : ci * DG + db * 128 + 128],
                                xT_sb[:, ci * T + qc * 512 : ci * T + qc * 512 + 512],
                                start=(ci == 0),
                                stop=(ci == CI - 1),
                            )
                        nc.scalar.activation(
                            t_sb[:, db * T + qc * 512 : db * T + qc * 512 + 512],
                            ps[:],
                            ACT.Identity,
                            bias=b_sb[:, db : db + 1],
                        )

            # v in augmented layout [t, 4 heads x (64 dims + ones col)]:
            # Wv is host-padded with zero columns at the ones slots and the
            # constant 1.0 comes from a K=1 bias matmul, so every write into
            # v_sb is a contiguous [128, 260] copy.
            for tb in range(TB):
                ps = pps.tile([128, HL * VROW], F32, tag="vproj")
                for ci in range(CI):
                    nc.tensor.matmul(
                        ps[:],
                        xT_sb[:, ci * T + tb * 128 : ci * T + tb * 128 + 128],
                        wv_sb[:, ci * HL * VROW : (ci + 1) * HL * VROW],
                        start=(ci == 0),
                        stop=False,
                    )
                nc.tensor.matmul(
                    ps[:], ones_sb[0:1, :], vb_sb[:], start=False, stop=True
                )
                nc.vector.tensor_copy(
                    v_sb[:, tb * HL * VROW : (tb + 1) * HL * VROW], ps[:]
                )

        # -------- Phase 2+3+4: attention / AllGather / out-proj --------
        # interleaved per q-chunk so the collective overlaps compute
        dram = ctx.enter_context(tc.tile_pool(name="dram", bufs=1, space="DRAM"))
        with ExitStack() as p2:
            qk_ps = p2.enter_context(tc.tile_pool(name="qk_ps", bufs=2, space="PSUM"))
            av_ps = p2.enter_context(tc.tile_pool(name="av_ps", bufs=1, space="PSUM"))
            bc_ps = p2.enter_context(tc.tile_pool(name="bc_ps", bufs=1, space="PSUM"))
            wo_ps = p2.enter_context(tc.tile_pool(name="wo_ps", bufs=1, space="PSUM"))
            expp = p2.enter_context(tc.tile_pool(name="expp", bufs=6))
            nrm = p2.enter_context(tc.tile_pool(name="nrm", bufs=2))
            outp = p2.enter_context(tc.tile_pool(name="outp", bufs=3))

            def emit_wo_tb(tb):
                wp = wo_ps.tile([128, DG], F32, tag="wo", name=f"wp{tb}")
                for ci in range(CI):
                    nc.tensor.matmul(
                        wp[:],
                        g_sb[:, ci * T + tb * 128 : ci * T + tb * 128 + 128],
                        wo_sb[:, ci * DG : (ci + 1) * DG],
                        start=(ci == 0),
                        stop=False,
                    )
                nc.tensor.matmul(
                    wp[:], ones_sb[0:1, :], bfin_sb[:], start=False, stop=True
                )
                ot = outp.tile([128, DG], F32, tag="out", name=f"ot{tb}")
                nc.vector.tensor_copy(ot[:], wp[:])
                nc.sync.dma_start(out_ext[tb * 128 : (tb + 1) * 128, :], ot[:])

            qc_order = [QC - 1 - i for i in range(QC)]
            prev_qc = None
            for qc in qc_order:
                nkb = 4 * (qc + 1)
                for hp in range(2):
                    h0, h1 = 2 * hp, 2 * hp + 1
                    dstrip = nrm.tile([65, 2 * 512], F32, tag="dstrip", name="ds")
                    q0 = qT_sb[0:64, hp * T + qc * 512 : hp * T + qc * 512 + 512]
                    q1 = qT_sb[64:128, hp * T + qc * 512 : hp * T + qc * 512 + 512]
                    av0 = av_ps.tile([VROW, 512], F32, tag="av0", name="av0")
                    av1 = av_ps.tile([VROW, 512], F32, tag="av1", name="av1")
                    LAG = 2
                    ets = {}

                    def emit_av(kb):
                        r = kb - 4 * qc
                        c0 = 128 * r if r > 0 else 0
                        e0, e1 = ets.pop(kb)
                        for h, avp, et in ((h0, av0, e0), (h1, av1, e1)):
                            nc.tensor.matmul(
                                avp[:, c0:512],
                                v_sb[
                                    :,
                                    kb * HL * VROW + h * VROW : kb * HL * VROW
                                    + (h + 1) * VROW,
                                ],
                                et[:, c0:512],
                                start=(kb == 0),
                                stop=(kb == nkb - 1),
                            )

                    for kb in range(nkb + LAG):
                        if kb < nkb:
                            r = kb - 4 * qc
                            c0 = 128 * r if r > 0 else 0
                            # two heads ride different PE row groups: their
                            # matmuls run concurrently and each LDWEIGHTS
                            # overlaps the sibling's matmul
                            qkp0 = qk_ps.tile([128, 512], F32, tag="qk0", name="qkp0")
                            qkp1 = qk_ps.tile([128, 512], F32, tag="qk1", name="qkp1")
                            nc.tensor.matmul(
                                qkp0[:, c0:512],
                                kT_sb[
                                    0:64,
                                    hp * T + kb * 128 : hp * T + kb * 128 + 128,
                                ],
                                q0[:, c0:512],
                                start=True,
                                stop=True,
                            )
                            nc.tensor.matmul(
                                qkp1[:, c0:512],
                                kT_sb[
                                    64:128,
                                    hp * T + kb * 128 : hp * T + kb * 128 + 128,
                                ],
                                q1[:, c0:512],
                                start=True,
                                stop=True,
                            )
                            e0 = expp.tile([128, 512], F16, tag="exp", name="e0")
                            e1 = expp.tile([128, 512], F16, tag="exp", name="e1")
                            ets[kb] = (e0, e1)
                            for h, qkp, et in ((h0, qkp0, e0), (h1, qkp1, e1)):
                                nc.scalar.activation(
                                    et[:, c0:512],
                                    qkp[:, c0:512],
                                    ACT.Exp,
                                    bias=alibi_sb[:, h * KB + kb : h * KB + kb + 1],
                                    scale=float(D) ** -0.5,
                                )
                                if r >= 0:
                                    nc.vector.tensor_tensor(
                                        et[:, c0 : c0 + 128],
                                        et[:, c0 : c0 + 128],
                                        tri_sb[:],
                                        AluOpType.mult,
                                    )
                        if kb >= LAG:
                            emit_av(kb - LAG)

                    # denominators + unnormalized numerators, then one row
                    # block of the previous chunk's out projection per head
                    for hl, avp in ((0, av0), (1, av1)):
                        d_sl = dstrip[64:65, hl * 512 : (hl + 1) * 512]
                        a_sl = attn_sb[
                            hl * 64 : hl * 64 + 64,
                            hp * T + qc * 512 : hp * T + qc * 512 + 512,
                        ]
                        nc.vector.tensor_copy(d_sl, avp[D : D + 1, :])
                        nc.vector.tensor_copy(a_sl, avp[0:D, :])
                        if prev_qc is not None:
                            emit_wo_tb(4 * prev_qc + 2 * hp + hl)

                    # per-pair batched normalization
                    rall = nrm.tile([2, 512], F32, tag="rall")
                    nc.sync.dma_start(
                        rall[:],
                        dstrip[64:65, :].rearrange("p (h x) -> p h x", h=2),
                    )
                    rrec = nrm.tile([2, 512], F32, tag="rrec")
                    nc.vector.reciprocal(rrec[:], rall[:])
                    rr16 = nrm.tile([2, 512], F16, tag="rr16")
                    nc.vector.tensor_copy(rr16[:], rrec[:])
                    bcp = bc_ps.tile([128, 512], F32, tag="bc")
                    nc.tensor.matmul(
                        bcp[:], sel_sb[0:2, :], rr16[:], start=True, stop=True
                    )
                    bcs = nrm.tile([128, 512], F16, tag="bcs")
                    nc.vector.tensor_copy(bcs[:], bcp[:])
                    a_pair = attn_sb[
                        :, hp * T + qc * 512 : hp * T + qc * 512 + 512
                    ]
                    nc.vector.tensor_tensor(a_pair, a_pair, bcs[:], AluOpType.mult)

                    # half-chunk AllGather (this head pair only) across the
                    # 4-core batch group; fires mid-chunk for deep overlap
                    attn_dram = dram.tile(
                        [128, 512], F16, tag=f"ad{qc}_{hp}", name=f"ad{qc}_{hp}"
                    )
                    gathered = dram.tile(
                        [HG * 128, 512], F16, tag=f"gd{qc}_{hp}", name=f"gd{qc}_{hp}"
                    )
                    nc.sync.dma_start(attn_dram[:], a_pair)
                    nc.gpsimd.collective_compute(
                        "AllGather",
                        mybir.AluOpType.bypass,
                        replica_groups=REPLICA_GROUPS,
                        ins=[attn_dram[:].opt()],
                        outs=[gathered[:].opt()],
                    )
                    for g in range(HG):
                        ci = 2 * g + hp
                        nc.sync.dma_start(
                            g_sb[:, ci * T + qc * 512 : ci * T + qc * 512 + 512],
                            gathered[g * 128 : (g + 1) * 128, :],
                        )
                prev_qc = qc
            for tb in range(4 * prev_qc, 4 * prev_qc + 4):
                emit_wo_tb(tb)

    nc.compile()
    return nc


def _get_compiled():
    global _COMPILED
    if _COMPILED is None:
        _COMPILED = _build()
    return _COMPILED


def _make_in_maps(x, Wq, bq, Wk, bk, Wv, bv, Wo, bo):
    slopes = _alibi_slopes(H)
    ones128 = np.ones((128, 128), dtype=np.float16)
    # tri[p, f] = 1 where key-offset p <= q-offset f (causal keep region)
    tri = np.triu(np.ones((128, 128), dtype=np.float16))
    sel = np.zeros((2, 128), dtype=np.float16)
    sel[0, 0:64] = 1.0
    sel[1, 64:128] = 1.0
    vbias = np.zeros((1, HL * VROW), dtype=np.float16)
    for h in range(HL):
        vbias[0, h * VROW + D] = 1.0
    in_maps = []
    for c in range(N_CORES):
        b, hg = divmod(c, HG)
        sl = slice(hg * DG, (hg + 1) * DG)
        alibi = np.empty((128, HL * KB), dtype=np.float32)
        p = np.arange(128, dtype=np.float32)[:, None]
        for h in range(HL):
            s = slopes[hg * HL + h]
            kbs = np.arange(KB, dtype=np.float32)[None, :]
            alibi[:, h * KB : (h + 1) * KB] = -s * (128.0 * kbs + p) - LN_SHIFT
        bfin = (bv @ Wo[:, sl] + bo[sl]).astype(np.float16).reshape(1, DG)
        wv_pad = np.zeros((C, HL * VROW), dtype=np.float16)
        for h in range(HL):
            wv_pad[:, h * VROW : h * VROW + D] = Wv[:, hg * DG + h * D : hg * DG + (h + 1) * D].astype(np.float16)
        in_maps.append(
            {
                "xT": np.ascontiguousarray(x[b].T).astype(np.float16),
                "wq": np.ascontiguousarray(Wq[:, sl]).astype(np.float16),
                "wk": np.ascontiguousarray(Wk[:, sl]).astype(np.float16),
                "wv": wv_pad,
                "wo": np.ascontiguousarray(Wo[:, sl]).astype(np.float16),
                "bq2": np.ascontiguousarray(bq[sl].reshape(2, 128)).astype(np.float32),
                "bk2": np.ascontiguousarray(bk[sl].reshape(2, 128)).astype(np.float32),
                "bfin": bfin,
                "alibi": alibi,
                "ones_": ones128,
                "tri": tri,
                "vb": vbias,
                "sel": sel,
            }
        )
    return in_maps


def kernel(x, Wq, bq, Wk, bk, Wv, bv, Wo, bo):
    global last_exec_time_ns, last_trace_path
    x = np.asarray(x, dtype=np.float32)
    Wq = np.asarray(Wq, dtype=np.float32)
    bq = np.asarray(bq, dtype=np.float32)
    Wk = np.asarray(Wk, dtype=np.float32)
    bk = np.asarray(bk, dtype=np.float32)
    Wv = np.asarray(Wv, dtype=np.float32)
    bv = np.asarray(bv, dtype=np.float32)
    Wo = np.asarray(Wo, dtype=np.float32)
    bo = np.asarray(bo, dtype=np.float32)

    from concourse import bass_utils

    trace = bool(os.environ.get("BASS_KERNEL_TRACE"))
    kwargs = {}
    if trace:
        try:
            import sys
            import types

            import antenv

            if "antenv.axon_hooks" not in sys.modules:
                hooks = types.ModuleType("antenv.axon_hooks")
                _h = [None]
                hooks.set_axon_ntff_profile_hook = lambda fn: _h.__setitem__(0, fn)
                hooks.get_axon_ntff_profile_hook = lambda: _h[0]
                sys.modules["antenv.axon_hooks"] = hooks
                antenv.axon_hooks = hooks
                from trn_agent_boot.trn_boot import _ntff_profile_via_ctypes

                hooks.set_axon_ntff_profile_hook(
                    _ntff_profile_via_ctypes("/opt/axon/libaxon_pjrt.so")
                )
            bass_utils.upload_artifacts = lambda tmpdir: "local://" + str(tmpdir)
            kwargs = {"trace": True, "tmpdir": os.environ.get("BASS_KERNEL_TRACE_DIR")}
        except Exception as e:  # pragma: no cover
            print(f"trace setup failed ({e}); running untraced")
            trace = False

    nc = _get_compiled()
    in_maps = _make_in_maps(x, Wq, bq, Wk, bk, Wv, bv, Wo, bo)
    res = bass_utils.run_bass_kernel_spmd(
        nc, in_maps, core_ids=list(range(N_CORES)), **kwargs
    )
    if trace:
        last_exec_time_ns = res.exec_time_ns
        if res.instructions_and_trace is not None:
            last_trace_path = res.instructions_and_trace[1]

    out = np.empty((B, T, C), dtype=np.float32)
    for c in range(N_CORES):
        b, hg = divmod(c, HG)
        out[b, :, hg * DG : (hg + 1) * DG] = res.results[c]["out"]
    return out
